# revision 26
# baseline (speedup 1.0000x reference)
# Block-circulant linear kernel for Trainium2 (raw Bass/Bacc, 8-core SPMD).
#
# y[b, 16m+p] = sum_{n,q} blocks[(m-n)%512, p, q] * x[b, 16n+q]
#
# Strategy: shard the output block axis m across 8 cores (64 block-rows per
# core). Per core the implied 8192x1024 weight slice tiles into 64x8 grid of
# 128x128 tiles with only 64 DISTINCT tiles G_s, s = (t-c) mod 64 (t = output
# 128-row tile, c = contraction 128-row chunk). The whole per-core compute is
# 71 accumulating matmuls (64 stationary loads) into one PSUM bank
# [128 (mm,p), 256 (t',b)]:
#   position v = 0..56 ("singles", s = v-56):  acc[:,0:256]   += G.T @ xt[:,32v:32v+256]
#   position v = 57..63 ("pairs",  j = v-56):  acc[:,256-32j:] += G.T @ xt[:,0:32j]
#                                              acc[:,0:256-32j]+= G.T @ xt[:,32(56+j):2048]
# xt is the reversed (c' = 63-c) x-transpose so both streams are consumed in
# strictly ascending column order; the psum t axis is flipped (t' = 7-t).
#
# This file intentionally avoids TileContext: the Tile/Bacc auto-dependency
# machinery allocates hundreds of event semaphores whose end-of-kernel
# teardown (~7.4us) plus per-matmul event preludes dominated the previous
# 27us version. Raw engine programs with 7 manually managed semaphores and
# at most one wait per instruction keep the teardown to two range-clears.
#
# Schedule: 13 input DMAs alternate across the two HWDGE queues (SP, ACT) in
# consumption order (small chunks first so the PE stream starts early); the
# PE warms up with throwaway matmuls (clock ramp 0.65->2.4GHz needs ~3us of
# continuous busy) sized to cover the ramp and the first chunks' arrival.
# Measured (8-core HW): ~24.4-25.4us vs 27.1us for the TileContext version.
import os
import numpy as np

B = 32
NB = 512          # number of 16x16 blocks
NCORES = 8
MBLK = NB // NCORES   # 64 output block-rows per core
NV = 64               # stationary tile positions
NW_WARM = int(os.environ.get("KWARM", "15"))   # warm-up matmuls

# Input chunks: (kind, col_lo, col_hi, queue); queue 0=SP-HWDGE, 1=ACT-HWDGE,
# 2=Pool-SWDGE. xt cols in [0,2048), bq cols in [0,8192); bytes = 256*(hi-lo).
# Position v consumes bq cols [128v,128v+128) and xt cols [32v, 32v+256).
# Sizes ramp up: aggregate DMA supply (~300GB/s) barely exceeds full-clock PE
# consumption (~290GB/s), so the stream start T0 (set by NW_WARM) is tuned to
# just trail the arrival curve; small early chunks keep deadlines smooth.
ITEMS = [
    ("xt", 0, 512, 2), ("bq", 0, 512, 0), ("bq", 512, 1024, 1),
    ("bq", 1024, 2048, 0), ("xt", 512, 1024, 2), ("bq", 2048, 3072, 1),
    ("bq", 3072, 4096, 0), ("xt", 1024, 2048, 2), ("bq", 4096, 5120, 1),
    ("bq", 5120, 6144, 0), ("bq", 6144, 7168, 1), ("bq", 7168, 7936, 0),
    ("bq", 7936, 8192, 1),
]

_cached = {}
_last_results = None  # BassKernelResults of the most recent run (for profiling)


def _item_deadline(kind, lo):
    """First position v that reads any element of this chunk."""
    if kind == "bq":
        return lo // 128
    return 0 if lo < 256 else (lo - 256) // 32 + 1


def _matmul_plan():
    """[(v, psum_lo, psum_hi, xt_lo, xt_hi, start, stop)] in emission order."""
    plan = []
    for v in range(NV):
        if v <= 56:
            plan.append((v, 0, 256, 32 * v, 32 * v + 256, v == 0, False))
        else:
            j = v - 56
            plan.append((v, 256 - 32 * j, 256, 0, 32 * j, False, False))
            plan.append((v, 0, 256 - 32 * j, 32 * (56 + j), 2048, False, v == NV - 1))
    return plan


def _build_raw():
    import concourse.bacc as bacc
    import concourse.mybir as mybir

    f16 = mybir.dt.float16
    f32 = mybir.dt.float32
    nc = bacc.Bacc("TRN2", target_bir_lowering=False, debug=False, num_devices=NCORES)
    xt_d = nc.declare_dram_parameter("xt", [128, 2048], f16, isOutput=False)
    bq_d = nc.declare_dram_parameter("bq", [128, 8192], f16, isOutput=False)
    out_d = nc.declare_dram_parameter("out", [128, 256], f32, isOutput=True)

    xt = nc.alloc_sbuf_tensor("xt_sb", [128, 2048], f16)
    bq = nc.alloc_sbuf_tensor("bq_sb", [128, 8192], f16)
    out_sb = nc.alloc_sbuf_tensor("out_sb", [128, 256], f32)
    warm = nc.alloc_sbuf_tensor("warm_sb", [128, 256], f16)
    acc = nc.alloc_psum_tensor("acc", [128, 256], f32)
    warm_ps = nc.alloc_psum_tensor("warm_ps", [128, 256], f32)

    # One semaphore per DMA: a HWDGE queue fans descriptors over 16 rings
    # whose completions interleave across consecutive DMAs, so a cumulative
    # per-queue count can reach 16*k before the k-th chunk is fully resident.
    s_dma = [nc.alloc_semaphore(f"s_dma{i}") for i in range(len(ITEMS))]
    s_pe = nc.alloc_semaphore("s_pe")
    s_cp1 = nc.alloc_semaphore("s_cp1")
    s_cp2 = nc.alloc_semaphore("s_cp2")
    s_out1 = nc.alloc_semaphore("s_out1")
    s_out2 = nc.alloc_semaphore("s_out2")
    all_sems = s_dma + [s_pe, s_cp1, s_cp2, s_out1, s_out2]

    queues = [nc.sync, nc.scalar, nc.gpsimd]
    waits_at = {}
    for idx, (kind, lo, hi, q) in enumerate(ITEMS):
        src, dst = (xt_d, xt) if kind == "xt" else (bq_d, bq)
        queues[q].dma_start(dst[:, lo:hi], src[:, lo:hi]).then_inc(s_dma[idx], 16)
        # Wait one position EARLY: a wait attached to matmul v lands on the
        # MATMUL after lowering, so position v+1's LDWEIGHTS (which pipelines
        # under MATMUL v) is fenced by it, but position v's own LDWEIGHTS is
        # not. Shifting each chunk's wait to the previous position closes
        # that prefetch race for every position except v=0, which instead
        # gets an explicit nop to absorb the wait ahead of its LDWEIGHTS.
        v_need = max(0, min(_item_deadline(kind, lo), NV - 1) - 1)
        waits_at.setdefault(v_need, []).append((s_dma[idx], 16))

    # PE: warm-up on uninitialized SBUF (results discarded) keeps the engine
    # continuously busy through the clock ramp while the first chunks stream.
    for w in range(NW_WARM):
        nc.tensor.matmul(
            warm_ps[:, 0:256], warm[:, 0:128], warm[:, 0:256],
            start=(w == 0), stop=(w == NW_WARM - 1), skip_group_check=True,
        )

    last_inst = None
    plan = _matmul_plan()
    seen_v = set()
    for (v, p_lo, p_hi, x_lo, x_hi, start, stop) in plan:
        if v not in seen_v:
            seen_v.add(v)
            for (sem, thr) in waits_at.get(v, []):
                nc.tensor.wait_ge(sem, thr)
                if v == 0:
                    nc.tensor.nop()
        last_inst = nc.tensor.matmul(
            acc[:, p_lo:p_hi], bq[:, 128 * v:128 * v + 128], xt[:, x_lo:x_hi],
            start=start, stop=stop, skip_group_check=True,
        )
    last_inst.then_inc(s_pe, 1)

    # DVE: psum -> sbuf in halves; each half leaves on its own HWDGE queue so
    # the two descriptor-gens and transfers overlap.
    nc.vector.wait_ge(s_pe, 1)
    nc.vector.tensor_copy(out_sb[:, 0:128], acc[:, 0:128]).then_inc(s_cp1, 1)
    nc.vector.tensor_copy(out_sb[:, 128:256], acc[:, 128:256]).then_inc(s_cp2, 1)
    nc.sync.wait_ge(s_cp1, 1)
    nc.sync.dma_start(out_d[:, 0:128], out_sb[:, 0:128]).then_inc(s_out1, 16)
    nc.scalar.wait_ge(s_cp2, 1)
    nc.scalar.dma_start(out_d[:, 128:256], out_sb[:, 128:256]).then_inc(s_out2, 16)

    # teardown: leave every semaphore zeroed for the next execution.
    nc.gpsimd.wait_ge(s_out1, 16)
    nc.gpsimd.wait_ge(s_out2, 16)
    nums = sorted(s.num for s in all_sems)
    if nums == list(range(nums[0], nums[-1] + 1)):
        rng = range(nums[0], nums[-1] + 1)
        nc.gpsimd.dma_reset(rng)
        nc.gpsimd.sem_clear(rng)
    else:
        for s in all_sems:
            nc.gpsimd.dma_reset(range(s.num, s.num + 1))
            nc.gpsimd.sem_clear(range(s.num, s.num + 1))
    nc.compile()
    return nc


def _get_program():
    if "raw" not in _cached:
        _cached["raw"] = _build_raw()
    return _cached["raw"]


def _prep_inputs(x, blocks):
    """Host-side layout prep (pure numpy reshuffles of the small inputs)."""
    x = np.ascontiguousarray(np.asarray(x), dtype=np.float32)
    blocks = np.ascontiguousarray(np.asarray(blocks), dtype=np.float32)
    # xt[(ni*16+q), c'*32+b] = x[b, 128*(63-c') + 16*ni + q]
    xt = x.T.reshape(64, 128, 32).transpose(1, 0, 2)[:, ::-1, :].reshape(128, 2048)
    xt16 = np.ascontiguousarray(xt.astype(np.float16))
    u = np.arange(NV * 8)
    ni = np.arange(8)
    in_maps = []
    for k in range(NCORES):
        m0 = k * MBLK
        # bq[(ni,q), (8v+mm)*16+p] = blocks[(m0 - 448 + 8v + mm - ni) % 512, p, q]
        idx = (m0 - 448 + u[None, :] - ni[:, None]) % NB      # [8, 512]
        bigq = blocks[idx]                                     # [8, 512, p, q]
        bigq = bigq.transpose(0, 3, 1, 2).reshape(128, NV * 128)  # [(ni,q),(u,p)]
        in_maps.append(
            {"xt": xt16, "bq": np.ascontiguousarray(bigq.astype(np.float16))}
        )
    return in_maps


def _assemble(results):
    y = np.empty((B, NB * 16), dtype=np.float32)
    for k in range(NCORES):
        o = np.asarray(results[k]["out"])  # [128 (mm,p), 256 (t',b)], t = 7-t'
        y[:, 1024 * k: 1024 * (k + 1)] = (
            o.reshape(128, 8, 32)[:, ::-1, :].transpose(2, 1, 0).reshape(32, 1024)
        )
    return y


def emulate(x, blocks):
    """Numpy emulation of the exact per-core AP slicing/matmul schedule."""
    in_maps = _prep_inputs(x, blocks)
    results = []
    for k in range(NCORES):
        xt = in_maps[k]["xt"].astype(np.float32)
        bq = in_maps[k]["bq"].astype(np.float32)
        acc = np.zeros((128, 256), dtype=np.float32)
        for (v, p_lo, p_hi, x_lo, x_hi, start, stop) in _matmul_plan():
            lhsT = bq[:, 128 * v:128 * v + 128]
            acc[:, p_lo:p_hi] += lhsT.T @ xt[:, x_lo:x_hi]
        results.append({"out": acc})
    return _assemble(results)


def kernel(x, blocks):
    global _last_results
    from concourse.bass_utils import run_bass_kernel_spmd

    nc = _get_program()
    in_maps = _prep_inputs(x, blocks)
    res = run_bass_kernel_spmd(nc, in_maps, list(range(NCORES)))
    _last_results = res
    return _assemble(res.results)


# revision 28
# speedup vs baseline: 1.0274x; 1.0274x over previous
# Block-circulant linear kernel for Trainium2 (raw Bass/Bacc, 8-core SPMD).
#
# y[b, 16m+p] = sum_{n,q} blocks[(m-n)%512, p, q] * x[b, 16n+q]
#
# Strategy: shard the output block axis m across 8 cores (64 block-rows per
# core). Per core the implied 8192x1024 weight slice tiles into 64x8 grid of
# 128x128 tiles with only 64 DISTINCT tiles G_s, s = (t-c) mod 64 (t = output
# 128-row tile, c = contraction 128-row chunk). The whole per-core compute is
# 71 accumulating matmuls (64 stationary loads) into one PSUM bank
# [128 (mm,p), 256 (t',b)]:
#   position v = 0..56 ("singles", s = v-56):  acc[:,0:256]   += G.T @ xt[:,32v:32v+256]
#   position v = 57..63 ("pairs",  j = v-56):  acc[:,256-32j:] += G.T @ xt[:,0:32j]
#                                              acc[:,0:256-32j]+= G.T @ xt[:,32(56+j):2048]
# xt is the reversed (c' = 63-c) x-transpose so both streams are consumed in
# strictly ascending column order; the psum t axis is flipped (t' = 7-t).
#
# This file intentionally avoids TileContext: the Tile/Bacc auto-dependency
# machinery allocates hundreds of event semaphores whose end-of-kernel
# teardown (~7.4us) plus per-matmul event preludes dominated the previous
# 27us version. Raw engine programs with 7 manually managed semaphores and
# at most one wait per instruction keep the teardown to two range-clears.
#
# Schedule: 13 input DMAs alternate across the two HWDGE queues (SP, ACT) in
# consumption order (small chunks first so the PE stream starts early); the
# PE warms up with throwaway matmuls (clock ramp 0.65->2.4GHz needs ~3us of
# continuous busy) sized to cover the ramp and the first chunks' arrival.
# Measured (8-core HW): ~24.4-25.4us vs 27.1us for the TileContext version.
import os
import numpy as np

B = 32
NB = 512          # number of 16x16 blocks
NCORES = 8
MBLK = NB // NCORES   # 64 output block-rows per core
NV = 64               # stationary tile positions
NW_WARM = int(os.environ.get("KWARM", "16"))   # warm-up matmuls

# Input chunks: (kind, col_lo, col_hi, queue); queue 0=SP-HWDGE, 1=ACT-HWDGE,
# 2=Pool-SWDGE. xt cols in [0,2048), bq cols in [0,8192); bytes = 256*(hi-lo).
# Position v consumes bq cols [128v,128v+128) and xt cols [32v, 32v+256).
# Sizes ramp up: aggregate DMA supply (~300GB/s) barely exceeds full-clock PE
# consumption (~290GB/s), so the stream start T0 (set by NW_WARM) is tuned to
# just trail the arrival curve; small early chunks keep deadlines smooth.
ITEMS = (
    [("xt", 0, 512, 0), ("bq", 0, 512, 1), ("bq", 512, 1024, 0),
     ("xt", 512, 1024, 1)]
    + [("bq", 1024 + 512 * i, 1536 + 512 * i, i % 2) for i in range(6)]
    + [("xt", 1024, 2048, 2)]
    + [("bq", 4096 + 512 * i, 4608 + 512 * i, (i + 1) % 2) for i in range(8)]
)

_cached = {}
_last_results = None  # BassKernelResults of the most recent run (for profiling)


def _item_deadline(kind, lo):
    """First position v that reads any element of this chunk."""
    if kind == "bq":
        return lo // 128
    return 0 if lo < 256 else (lo - 256) // 32 + 1


def _matmul_plan():
    """[(v, psum_lo, psum_hi, xt_lo, xt_hi, start, stop)] in emission order."""
    plan = []
    for v in range(NV):
        if v <= 56:
            plan.append((v, 0, 256, 32 * v, 32 * v + 256, v == 0, False))
        else:
            j = v - 56
            plan.append((v, 256 - 32 * j, 256, 0, 32 * j, False, False))
            plan.append((v, 0, 256 - 32 * j, 32 * (56 + j), 2048, False, v == NV - 1))
    return plan


def _build_raw():
    import concourse.bacc as bacc
    import concourse.mybir as mybir

    f16 = mybir.dt.float16
    f32 = mybir.dt.float32
    nc = bacc.Bacc("TRN2", target_bir_lowering=False, debug=False, num_devices=NCORES)
    xt_d = nc.declare_dram_parameter("xt", [128, 2048], f16, isOutput=False)
    bq_d = nc.declare_dram_parameter("bq", [128, 8192], f16, isOutput=False)
    out_d = nc.declare_dram_parameter("out", [128, 256], f32, isOutput=True)

    xt = nc.alloc_sbuf_tensor("xt_sb", [128, 2048], f16)
    bq = nc.alloc_sbuf_tensor("bq_sb", [128, 8192], f16)
    out_sb = nc.alloc_sbuf_tensor("out_sb", [128, 256], f32)
    warm = nc.alloc_sbuf_tensor("warm_sb", [128, 256], f16)
    acc = nc.alloc_psum_tensor("acc", [128, 256], f32)
    warm_ps = nc.alloc_psum_tensor("warm_ps", [128, 256], f32)

    # One semaphore per DMA: a HWDGE queue fans descriptors over 16 rings
    # whose completions interleave across consecutive DMAs, so a cumulative
    # per-queue count can reach 16*k before the k-th chunk is fully resident.
    s_dma = [nc.alloc_semaphore(f"s_dma{i}") for i in range(len(ITEMS))]
    s_pe = nc.alloc_semaphore("s_pe")
    s_cp1 = nc.alloc_semaphore("s_cp1")
    s_cp2 = nc.alloc_semaphore("s_cp2")
    s_out1 = nc.alloc_semaphore("s_out1")
    s_out2 = nc.alloc_semaphore("s_out2")
    all_sems = s_dma + [s_pe, s_cp1, s_cp2, s_out1, s_out2]

    queues = [nc.sync, nc.scalar, nc.gpsimd]
    waits_at = {}
    for idx, (kind, lo, hi, q) in enumerate(ITEMS):
        src, dst = (xt_d, xt) if kind == "xt" else (bq_d, bq)
        queues[q].dma_start(dst[:, lo:hi], src[:, lo:hi]).then_inc(s_dma[idx], 16)
        # Wait one position EARLY: a wait attached to matmul v lands on the
        # MATMUL after lowering, so position v+1's LDWEIGHTS (which pipelines
        # under MATMUL v) is fenced by it, but position v's own LDWEIGHTS is
        # not. Shifting each chunk's wait to the previous position closes
        # that prefetch race for every position except v=0, which instead
        # gets an explicit nop to absorb the wait ahead of its LDWEIGHTS.
        v_need = max(0, min(_item_deadline(kind, lo), NV - 1) - 1)
        waits_at.setdefault(v_need, []).append((s_dma[idx], 16))

    # PE: warm-up on uninitialized SBUF (results discarded) keeps the engine
    # continuously busy through the clock ramp while the first chunks stream.
    for w in range(NW_WARM):
        nc.tensor.matmul(
            warm_ps[:, 0:256], warm[:, 0:128], warm[:, 0:256],
            start=(w == 0), stop=(w == NW_WARM - 1), skip_group_check=True,
        )

    last_inst = None
    plan = _matmul_plan()
    seen_v = set()
    for (v, p_lo, p_hi, x_lo, x_hi, start, stop) in plan:
        if v not in seen_v:
            seen_v.add(v)
            for (sem, thr) in waits_at.get(v, []):
                nc.tensor.wait_ge(sem, thr)
                if v == 0:
                    nc.tensor.nop()
        last_inst = nc.tensor.matmul(
            acc[:, p_lo:p_hi], bq[:, 128 * v:128 * v + 128], xt[:, x_lo:x_hi],
            start=start, stop=stop, skip_group_check=True,
        )
    last_inst.then_inc(s_pe, 1)

    # DVE: psum -> sbuf in halves; each half leaves on its own HWDGE queue so
    # the two descriptor-gens and transfers overlap.
    nc.vector.wait_ge(s_pe, 1)
    nc.vector.tensor_copy(out_sb[:, 0:128], acc[:, 0:128]).then_inc(s_cp1, 1)
    nc.vector.tensor_copy(out_sb[:, 128:256], acc[:, 128:256]).then_inc(s_cp2, 1)
    nc.sync.wait_ge(s_cp1, 1)
    nc.sync.dma_start(out_d[:, 0:128], out_sb[:, 0:128]).then_inc(s_out1, 16)
    nc.scalar.wait_ge(s_cp2, 1)
    nc.scalar.dma_start(out_d[:, 128:256], out_sb[:, 128:256]).then_inc(s_out2, 16)

    # teardown: leave every semaphore zeroed for the next execution.
    nc.gpsimd.wait_ge(s_out1, 16)
    nc.gpsimd.wait_ge(s_out2, 16)
    nums = sorted(s.num for s in all_sems)
    if nums == list(range(nums[0], nums[-1] + 1)):
        rng = range(nums[0], nums[-1] + 1)
        nc.gpsimd.dma_reset(rng)
        nc.gpsimd.sem_clear(rng)
    else:
        for s in all_sems:
            nc.gpsimd.dma_reset(range(s.num, s.num + 1))
            nc.gpsimd.sem_clear(range(s.num, s.num + 1))
    nc.compile()
    return nc


def _get_program():
    if "raw" not in _cached:
        _cached["raw"] = _build_raw()
    return _cached["raw"]


def _prep_inputs(x, blocks):
    """Host-side layout prep (pure numpy reshuffles of the small inputs)."""
    x = np.ascontiguousarray(np.asarray(x), dtype=np.float32)
    blocks = np.ascontiguousarray(np.asarray(blocks), dtype=np.float32)
    # xt[(ni*16+q), c'*32+b] = x[b, 128*(63-c') + 16*ni + q]
    xt = x.T.reshape(64, 128, 32).transpose(1, 0, 2)[:, ::-1, :].reshape(128, 2048)
    xt16 = np.ascontiguousarray(xt.astype(np.float16))
    u = np.arange(NV * 8)
    ni = np.arange(8)
    in_maps = []
    for k in range(NCORES):
        m0 = k * MBLK
        # bq[(ni,q), (8v+mm)*16+p] = blocks[(m0 - 448 + 8v + mm - ni) % 512, p, q]
        idx = (m0 - 448 + u[None, :] - ni[:, None]) % NB      # [8, 512]
        bigq = blocks[idx]                                     # [8, 512, p, q]
        bigq = bigq.transpose(0, 3, 1, 2).reshape(128, NV * 128)  # [(ni,q),(u,p)]
        in_maps.append(
            {"xt": xt16, "bq": np.ascontiguousarray(bigq.astype(np.float16))}
        )
    return in_maps


def _assemble(results):
    y = np.empty((B, NB * 16), dtype=np.float32)
    for k in range(NCORES):
        o = np.asarray(results[k]["out"])  # [128 (mm,p), 256 (t',b)], t = 7-t'
        y[:, 1024 * k: 1024 * (k + 1)] = (
            o.reshape(128, 8, 32)[:, ::-1, :].transpose(2, 1, 0).reshape(32, 1024)
        )
    return y


def emulate(x, blocks):
    """Numpy emulation of the exact per-core AP slicing/matmul schedule."""
    in_maps = _prep_inputs(x, blocks)
    results = []
    for k in range(NCORES):
        xt = in_maps[k]["xt"].astype(np.float32)
        bq = in_maps[k]["bq"].astype(np.float32)
        acc = np.zeros((128, 256), dtype=np.float32)
        for (v, p_lo, p_hi, x_lo, x_hi, start, stop) in _matmul_plan():
            lhsT = bq[:, 128 * v:128 * v + 128]
            acc[:, p_lo:p_hi] += lhsT.T @ xt[:, x_lo:x_hi]
        results.append({"out": acc})
    return _assemble(results)


def kernel(x, blocks):
    global _last_results
    from concourse.bass_utils import run_bass_kernel_spmd

    nc = _get_program()
    in_maps = _prep_inputs(x, blocks)
    res = run_bass_kernel_spmd(nc, in_maps, list(range(NCORES)))
    _last_results = res
    return _assemble(res.results)
